# revision 38
# baseline (speedup 1.0000x reference)
"""Trainium2 Bass kernel for nn_NNSensorResponse (histogram_binning).

Computes, for N=300000 electrons:
    h1 = relu(xy @ W1 + b1);  h2 = relu(h1 @ W2 + b2)
    r  = el * sigmoid(h2 @ W3 + b3)                      # [N, 64]
    g[n, t] = c * exp(-(t - z_n)^2 / 2)                  # [N, 1024]
    out = r.T @ g                                        # [64, 1024]

Strategy: shard electrons by z-range across 8 cores (128 ticks/core).
Within a core, electrons are bucketed into 16-tick blocks; the Gaussian
(sigma=1) is truncated to a 32-tick window per block (|d| >= 8 contributes
< 1.3e-14 relatively - far below fp32 resolution of the output).  Each
128-electron chunk contributes one accumulating matmul into a persistent
[64, 144] PSUM accumulator (144 = 128 core ticks + 8 margin each side).

Key trick: the whole Gaussian exponent (including the el_photons factor)
is assembled by a single K=3 matmul in block-relative coordinates:
    arg[e,u] = z'_e * t'_u  -  0.5 * t'_u^2  +  (-0.5 z'_e^2 + ln(c*el_e))
             = -0.5 (t'_u - z'_e)^2 + ln(c * el_e)
so  g = el * c * exp(-d^2/2)  is one PE op + one ACT exp per 8 chunks.

Per-core device pipeline, per group of 8 chunks (1024 electrons):
  - MLP: 2 batches of 512 electrons, hidden-on-partition layout:
      mm1: psum[128h,512e] = W1[2,128].T @ xyT[2,512]; relu(+b1) -> sbuf
      mm2: psum[128h,512e] = W2[128,128].T @ h1;       relu(+b2) -> sbuf
      (evictions alternate between ACT and DVE to balance engine load)
  - mm3 per chunk: psum[128e, 64s] slices stacked 8 wide -> [128, 512]
      sigmoid via tanh (same ACT table set as exp):
      ACT tanh(0.5*resp) -> sbuf; 0.5*t+0.5 in place (GPSIMD) => r-stack
  - Gaussian: per chunk a K=3 matmul (above) -> psum [128, 256] stack;
      one ACT exp (psum-src) -> g [128, 256]
  - einsum per chunk: acc[64, win:win+32] += r_chunk[128,64].T @ g_chunk[128,32]
Host gathers the 8 [64,144] partials into the full [64,1024] (overlaps sum).

Requires b3 == 0 (true for this problem's setup_inputs); asserted at runtime.
"""

import math

import ml_dtypes
import numpy as np

import concourse.bacc as bacc
import concourse.tile as tile
from concourse import mybir
from concourse.bass_utils import run_bass_kernel_spmd

N_CORES = 8
N_ELECTRONS = 300000
T_TICKS = 1024
S = 64          # sensors
H = 128         # hidden
CORE_TICKS = T_TICKS // N_CORES      # 128
BLOCK_TICKS = 16
BLOCKS = CORE_TICKS // BLOCK_TICKS   # 8
WIN = 32                              # per-block gaussian window (ticks)
MARGIN = 8                            # half-window margin
OUT_W = CORE_TICKS + 2 * MARGIN      # 144 output columns per core
CHUNK = 128                           # electrons per chunk (matmul K)
GROUP = 16                            # chunks per stacking group
BATCH = 1024                          # electrons per MLP batch
SPLIT = 512                           # relu2 eviction column split (ACT | DVE)
C_GAUSS = 0.3989422804                # 1/sqrt(2*pi)
NEG_BIG = -6.0e4                      # exponent for padded electrons (f16-safe)

FP = mybir.dt.float32
F16 = mybir.dt.float16
BF16 = mybir.dt.bfloat16
F32R = mybir.dt.float32r


def _build_nc(n_chunks_per_block, reps=1, n_stages=4):
    """Build + compile the per-core Bass program. n_chunks_per_block is a
    tuple of BLOCKS ints (shared across cores; data-dependent).  With
    reps>1 the full computation is repeated back-to-back (for benchmarking:
    amortizes per-dispatch launch overhead; every rep recomputes the output
    from DRAM inputs and rewrites y identically)."""
    C_tot = sum(n_chunks_per_block)
    assert C_tot % GROUP == 0
    M = C_tot * CHUNK
    n_groups = C_tot // GROUP

    # chunk index -> block id
    chunk_block = []
    for b, cb in enumerate(n_chunks_per_block):
        chunk_block.extend([b] * cb)

    nc = bacc.Bacc(None, target_bir_lowering=False)
    xyT = nc.dram_tensor("xyT", [2, M], BF16, kind="ExternalInput")
    zno = nc.dram_tensor("zno", [3, M], F16, kind="ExternalInput")
    tkr = nc.dram_tensor("tkr", [3, WIN], F16, kind="ExternalInput")
    w1 = nc.dram_tensor("w1", [2, H], BF16, kind="ExternalInput")
    w2 = nc.dram_tensor("w2", [H, H], BF16, kind="ExternalInput")
    w3 = nc.dram_tensor("w3", [H, S], F16, kind="ExternalInput")
    b1 = nc.dram_tensor("b1", [H, 1], FP, kind="ExternalInput")
    b2 = nc.dram_tensor("b2", [H, 1], FP, kind="ExternalInput")
    y = nc.dram_tensor("y", [S, OUT_W], FP, kind="ExternalOutput")

    with tile.TileContext(nc) as tc:
        with (
            tc.tile_pool(name="consts", bufs=1) as consts,
            tc.tile_pool(name="xyp", bufs=2) as xyp,
            tc.tile_pool(name="znop", bufs=2) as znop,
            tc.tile_pool(name="h1p", bufs=4) as h1p,
            tc.tile_pool(name="h2p", bufs=6) as h2p,
            tc.tile_pool(name="rp", bufs=6) as rp,
            tc.tile_pool(name="gp", bufs=6) as gp,
            tc.tile_pool(name="outp", bufs=2) as outp,
            tc.tile_pool(name="acc", bufs=1, space="PSUM") as accp,
            tc.tile_pool(name="mlppsum", bufs=2, space="PSUM") as mlppsum,
            tc.tile_pool(name="rdpsum", bufs=2, space="PSUM") as rdpsum,
            tc.tile_pool(name="dpsum", bufs=1, space="PSUM") as dpsum,
        ):
            # --- constants, loaded once ---
            w1_s = consts.tile([2, H], BF16)
            nc.sync.dma_start(out=w1_s[:], in_=w1[:])
            w2_s = consts.tile([H, H], BF16)
            nc.sync.dma_start(out=w2_s[:], in_=w2[:])
            w3_s = consts.tile([H, S], F16)
            nc.sync.dma_start(out=w3_s[:], in_=w3[:])
            b1_s = consts.tile([H, 1], FP)
            nc.sync.dma_start(out=b1_s[:], in_=b1[:])
            b2_s = consts.tile([H, 1], FP)
            nc.sync.dma_start(out=b2_s[:], in_=b2[:])
            tkr_s = consts.tile([3, WIN], F16)
            nc.sync.dma_start(out=tkr_s[:], in_=tkr[:])
            zeros_s = consts.tile([1, OUT_W], FP)
            nc.vector.memset(zeros_s[:], 0.0)
            zcol_s = consts.tile([CHUNK, 1], FP)
            nc.vector.memset(zcol_s[:], 0.0)

            def evict_relu(dst, src, bias, use_act):
                if use_act:
                    nc.scalar.activation(
                        dst, src, mybir.ActivationFunctionType.Relu, bias=bias
                    )
                else:
                    nc.vector.tensor_scalar(
                        out=dst, in0=src, scalar1=bias, scalar2=0.0,
                        op0=mybir.AluOpType.add, op1=mybir.AluOpType.max,
                    )

            # ---------- 5-stage software pipeline over groups ----------
            # S0(g): DMA xy, zno           S1(g): mm1, relu1
            # S2(g): mm2, relu2            S3(g): mm3 x8, tanh, rTS,
            #                                     d-mm x8, exp
            # S4(g): einsum x8
            # Each engine's dependencies are >= 1 iteration old, so the
            # per-engine instruction streams never head-of-line block.
            # `acc` and `st` are rebound per rep by the driver loop below;
            # the stage closures read the current binding at call time.
            acc = None
            st = {}  # per-group pipeline state

            DMA_G = 8  # groups fetched per input DMA pair

            def s0(g):
                if g % DMA_G != 0:
                    return
                ng = min(DMA_G, n_groups - g)
                e0 = g * GROUP * CHUNK
                w = ng * GROUP * CHUNK
                xy_t = xyp.tile([2, DMA_G * GROUP * CHUNK], BF16, tag="xy")
                nc.sync.dma_start(out=xy_t[:, 0:w], in_=xyT[:, e0:e0 + w])
                zno_t = znop.tile([3, DMA_G * GROUP * CHUNK], F16, tag="zno")
                nc.sync.dma_start(out=zno_t[:, 0:w], in_=zno[:, e0:e0 + w])
                for j in range(ng):
                    st[g + j] = {"xy": xy_t, "zno": zno_t,
                                 "off": j * GROUP * CHUNK}

            def s1(g):
                s = st[g]
                h1_s = h1p.tile([H, GROUP * CHUNK], BF16, tag="h1")
                s["h1"] = h1_s
                for sb in range(GROUP * CHUNK // BATCH):
                    h1_ps = mlppsum.tile([H, BATCH], FP, tag="mlp")
                    for half in range(2):
                        nc.tensor.matmul(
                            out=h1_ps[:, half * 512:(half + 1) * 512],
                            lhsT=w1_s[:],
                            rhs=s["xy"][:, s["off"] + sb * BATCH + half * 512:
                                        s["off"] + sb * BATCH + (half + 1) * 512],
                            start=True, stop=True,
                        )
                    evict_relu(h1_s[:, sb * BATCH:(sb + 1) * BATCH],
                               h1_ps[:], b1_s[:, 0:1], False)  # DVE

            def s2(g):
                s = st[g]
                h2_s = h2p.tile([H, GROUP * CHUNK], F16, tag="h2")
                s["h2"] = h2_s
                for sb in range(GROUP * CHUNK // BATCH):
                    h2_ps = mlppsum.tile([H, BATCH], FP, tag="mlp")
                    for half in range(2):
                        nc.tensor.matmul(
                            out=h2_ps[:, half * 512:(half + 1) * 512],
                            lhsT=w2_s[:],
                            rhs=s["h1"][:, sb * BATCH + half * 512:
                                        sb * BATCH + (half + 1) * 512],
                            start=True, stop=True,
                        )
                    # column-split eviction: ACT | DVE
                    o = sb * BATCH
                    evict_relu(h2_s[:, o:o + SPLIT], h2_ps[:, 0:SPLIT],
                               b2_s[:, 0:1], True)
                    evict_relu(h2_s[:, o + SPLIT:o + BATCH],
                               h2_ps[:, SPLIT:BATCH], b2_s[:, 0:1], False)
                s["h2"] = h2_s

            def s3(g):
                s = st[g]
                r_s = rp.tile([CHUNK, GROUP * S], F16, tag="r")
                for hh in range(2):
                    resp_ps = rdpsum.tile([CHUNK, GROUP * S // 2], FP, tag="rd")
                    for ci in range(GROUP // 2):
                        cc = hh * (GROUP // 2) + ci
                        nc.tensor.matmul(
                            out=resp_ps[:, ci * S:(ci + 1) * S],
                            lhsT=s["h2"][:, cc * CHUNK:(cc + 1) * CHUNK],
                            rhs=w3_s[:],
                            start=True, stop=True,
                        )
                    # sigmoid(x) = 0.5*tanh(0.5x) + 0.5; tanh on ACT (exp's
                    # table set), then the affine on the idle Pool engine.
                    half = r_s[:, hh * (GROUP // 2) * S:(hh + 1) * (GROUP // 2) * S]
                    nc.scalar.activation(
                        half, resp_ps[:], mybir.ActivationFunctionType.Tanh,
                        scale=0.5,
                    )
                    nc.gpsimd.tensor_scalar(
                        out=half, in0=half, scalar1=0.5, scalar2=0.5,
                        op0=mybir.AluOpType.mult, op1=mybir.AluOpType.add,
                    )
                d_ps = dpsum.tile([CHUNK, GROUP * WIN], FP, tag="d")
                for cc in range(GROUP):
                    last_pe = nc.tensor.matmul(
                        out=d_ps[:, cc * WIN:(cc + 1) * WIN],
                        lhsT=s["zno"][:, s["off"] + cc * CHUNK:
                                      s["off"] + (cc + 1) * CHUNK],
                        rhs=tkr_s[:],
                        start=True, stop=True,
                    )
                g_s = gp.tile([CHUNK, GROUP * WIN], F16, tag="g")
                nc.scalar.activation(
                    g_s[:], d_ps[:], mybir.ActivationFunctionType.Exp,
                    bias=zcol_s[:, 0:1],
                )
                s["r"] = r_s
                s["g"] = g_s
                s["last_pe"] = last_pe

            def s4(g):
                from concourse.tile_rust import add_dep_helper
                s = st.pop(g)
                first = True
                for cc in range(GROUP):
                    b = chunk_block[g * GROUP + cc]
                    w = b * BLOCK_TICKS
                    last = g == n_groups - 1 and cc == GROUP - 1
                    mm = nc.tensor.matmul(
                        out=acc[0:S, w:w + WIN],
                        lhsT=s["r"][:, cc * S:(cc + 1) * S],
                        rhs=s["g"][:, cc * WIN:(cc + 1) * WIN],
                        start=False, stop=last,
                        skip_group_check=True,
                    )
                    if first and (g + 2) in st and "last_pe" in st[g + 2]:
                        # order-only edge: keep einsum(g) after group g+2's
                        # matmul phase so PE never head-of-line blocks on
                        # the tanh/exp chain of group g.
                        add_dep_helper(
                            mm.ins, st[g + 2]["last_pe"].ins,
                            sync=False, reason="einsum skew",
                        )
                    first = False

            # einsum gets a 2-iteration skew (offset 5, not 4) so the
            # tanh -> Pool rTS chain of group g completes a full iteration
            # before PE reaches einsum(g) in its stream.
            stage_offsets = [(s0, 0), (s1, 1), (s2, 2), (s3, 3), (s4, 5)]
            stage_offsets = stage_offsets[:n_stages + 1]
            for _rep in range(reps):
                acc = accp.tile([S, OUT_W], FP, tag="acc")
                nc.tensor.matmul(
                    out=acc[:],
                    lhsT=zeros_s[0:1, 0:S],
                    rhs=zeros_s[0:1, 0:OUT_W],
                    start=True,
                    stop=False,
                    skip_group_check=True,
                )
                st = {}
                for it in range(n_groups + 6):
                    for fn, off in stage_offsets:
                        g = it - off
                        if 0 <= g < n_groups:
                            fn(g)

                out_sb = outp.tile([S, OUT_W], FP, tag="out")
                nc.vector.tensor_copy(out=out_sb[:], in_=acc[:])
                nc.sync.dma_start(out=y[:], in_=out_sb[:])

    nc.compile()
    return nc


_CACHE = {}


def _get_nc(n_chunks_per_block, reps=1, n_stages=4):
    key = (tuple(n_chunks_per_block), reps, n_stages)
    if key not in _CACHE:
        _CACHE[key] = _build_nc(key[0], reps=reps, n_stages=n_stages)
    return _CACHE[key]


def _prep_inputs(el_photons, xy_positions, z_positions):
    """Shard by z-range, bucket into 16-tick blocks, pad, build per-core
    device arrays."""
    el = np.asarray(el_photons, np.float32).reshape(-1)
    xy = np.asarray(xy_positions, np.float32)
    z = np.asarray(z_positions, np.float32).reshape(-1)

    core = np.clip((z // CORE_TICKS).astype(np.int64), 0, N_CORES - 1)
    zrel = z - core * CORE_TICKS
    block = np.clip((zrel // BLOCK_TICKS).astype(np.int64), 0, BLOCKS - 1)
    # block-relative z', and the per-electron exponent constant
    zp = (zrel - (block * BLOCK_TICKS + BLOCK_TICKS // 2)).astype(np.float32)
    wexp = np.where(
        el > 0,
        (-0.5 * zp.astype(np.float64) ** 2
         + np.log(np.maximum(el, 1e-45).astype(np.float64) * C_GAUSS)),
        NEG_BIG,
    ).astype(np.float32)

    counts = np.zeros((N_CORES, BLOCKS), np.int64)
    np.add.at(counts, (core, block), 1)
    cpb = np.ceil(counts.max(axis=0) / CHUNK).astype(np.int64)  # chunks per block
    C_tot = int(cpb.sum())
    pad_chunks = (-C_tot) % GROUP
    cpb[0] += pad_chunks
    C_tot += pad_chunks
    M = C_tot * CHUNK

    order = np.lexsort((block, core))  # stable sort by (core, block)
    el_o, xy_o, zp_o, w_o, blk_o, core_o = (
        el[order], xy[order], zp[order], wexp[order], block[order], core[order]
    )

    block_starts = np.concatenate(([0], np.cumsum(cpb)[:-1])) * CHUNK

    per_core = []
    for k in range(N_CORES):
        xyT_k = np.zeros((2, M), np.float32)
        zno_k = np.empty((3, M), np.float16)
        zno_k[0] = 0.0         # padded electrons: z' = 0
        zno_k[1] = -0.5
        zno_k[2] = NEG_BIG     # padded electrons contribute exp(-1e30) = 0
        sel = core_o == k
        xy_sel = xy_o[sel]
        zp_sel = zp_o[sel]
        w_sel = w_o[sel]
        blk_sel = blk_o[sel]
        bcounts = np.bincount(blk_sel, minlength=BLOCKS)
        src = 0
        for b in range(BLOCKS):
            nb = int(bcounts[b])
            dst = int(block_starts[b])
            xyT_k[:, dst:dst + nb] = xy_sel[src:src + nb].T
            zno_k[0, dst:dst + nb] = zp_sel[src:src + nb]
            zno_k[2, dst:dst + nb] = w_sel[src:src + nb]
            src += nb
        per_core.append((xyT_k.astype(ml_dtypes.bfloat16), zno_k))

    # tick rows, block-relative: t' in [-16, 16)
    tp = np.arange(WIN, dtype=np.float32) - (BLOCK_TICKS // 2 + MARGIN)
    tkr = np.stack([tp, tp * tp, np.ones(WIN, np.float32)]).astype(np.float16)
    return tuple(int(c) for c in cpb), per_core, tkr


def kernel(el_photons, xy_positions, z_positions, W1, b1, W2, b2, W3, b3):
    b3 = np.asarray(b3, np.float32)
    assert np.allclose(b3, 0.0), "kernel assumes b3 == 0"

    cpb, per_core, tkr = _prep_inputs(el_photons, xy_positions, z_positions)
    nc = _get_nc(cpb)

    shared = {
        "tkr": tkr,
        "w1": np.asarray(W1, np.float32).astype(__import__("ml_dtypes").bfloat16),
        "w2": np.asarray(W2, np.float32).astype(__import__("ml_dtypes").bfloat16),
        "w3": np.asarray(W3, np.float16),
        "b1": np.asarray(b1, np.float32).reshape(H, 1),
        "b2": np.asarray(b2, np.float32).reshape(H, 1),
    }
    in_maps = []
    for k in range(N_CORES):
        xyT_k, zno_k = per_core[k]
        in_maps.append({"xyT": xyT_k, "zno": zno_k, **shared})

    res = run_bass_kernel_spmd(nc, in_maps, core_ids=list(range(N_CORES)))

    out = np.zeros((S, T_TICKS), np.float64)
    for k in range(N_CORES):
        yk = res.results[k]["y"].astype(np.float64)
        lo = k * CORE_TICKS - MARGIN
        j0 = max(0, -lo)
        j1 = min(OUT_W, T_TICKS - lo)
        out[:, lo + j0:lo + j1] += yk[:, j0:j1]
    return out.astype(np.float32)



# revision 40
# speedup vs baseline: 1.0251x; 1.0251x over previous
"""Trainium2 Bass kernel for nn_NNSensorResponse (histogram_binning).

Computes, for N=300000 electrons:
    h1 = relu(xy @ W1 + b1);  h2 = relu(h1 @ W2 + b2)
    r  = el * sigmoid(h2 @ W3 + b3)                      # [N, 64]
    g[n, t] = c * exp(-(t - z_n)^2 / 2)                  # [N, 1024]
    out = r.T @ g                                        # [64, 1024]

Strategy: shard electrons by z-range across 8 cores (128 ticks/core).
Within a core, electrons are bucketed into 16-tick blocks; the Gaussian
(sigma=1) is truncated to a 32-tick window per block (|d| >= 8 contributes
< 1.3e-14 relatively - far below fp32 resolution of the output).  Each
128-electron chunk contributes one accumulating matmul into a persistent
[64, 144] PSUM accumulator (144 = 128 core ticks + 8 margin each side).

Key trick: the whole Gaussian exponent (including the el_photons factor)
is assembled by a single K=3 matmul in block-relative coordinates:
    arg[e,u] = z'_e * t'_u  -  0.5 * t'_u^2  +  (-0.5 z'_e^2 + ln(c*el_e))
             = -0.5 (t'_u - z'_e)^2 + ln(c * el_e)
so  g = el * c * exp(-d^2/2)  is one PE op + one ACT exp per 8 chunks.

Per-core device pipeline, per group of 8 chunks (1024 electrons):
  - MLP: 2 batches of 512 electrons, hidden-on-partition layout:
      mm1: psum[128h,512e] = W1[2,128].T @ xyT[2,512]; relu(+b1) -> sbuf
      mm2: psum[128h,512e] = W2[128,128].T @ h1;       relu(+b2) -> sbuf
      (evictions alternate between ACT and DVE to balance engine load)
  - mm3 per chunk: psum[128e, 64s] slices stacked 8 wide -> [128, 512]
      sigmoid via tanh (same ACT table set as exp):
      ACT tanh(0.5*resp) -> sbuf; 0.5*t+0.5 in place (GPSIMD) => r-stack
  - Gaussian: per chunk a K=3 matmul (above) -> psum [128, 256] stack;
      one ACT exp (psum-src) -> g [128, 256]
  - einsum per chunk: acc[64, win:win+32] += r_chunk[128,64].T @ g_chunk[128,32]
Host gathers the 8 [64,144] partials into the full [64,1024] (overlaps sum).

Requires b3 == 0 (true for this problem's setup_inputs); asserted at runtime.
"""

import math

import ml_dtypes
import numpy as np

import concourse.bacc as bacc
import concourse.tile as tile
from concourse import mybir
from concourse.bass_utils import run_bass_kernel_spmd

N_CORES = 8
N_ELECTRONS = 300000
T_TICKS = 1024
S = 64          # sensors
H = 128         # hidden
CORE_TICKS = T_TICKS // N_CORES      # 128
BLOCK_TICKS = 16
BLOCKS = CORE_TICKS // BLOCK_TICKS   # 8
WIN = 32                              # per-block gaussian window (ticks)
MARGIN = 8                            # half-window margin
OUT_W = CORE_TICKS + 2 * MARGIN      # 144 output columns per core
CHUNK = 128                           # electrons per chunk (matmul K)
GROUP = 16                            # chunks per stacking group
BATCH = 1024                          # electrons per MLP batch
SPLIT = 512                           # relu2 eviction column split (ACT | DVE)
C_GAUSS = 0.3989422804                # 1/sqrt(2*pi)
NEG_BIG = -6.0e4                      # exponent for padded electrons (f16-safe)

FP = mybir.dt.float32
F16 = mybir.dt.float16
BF16 = mybir.dt.bfloat16
F32R = mybir.dt.float32r


def _build_nc(n_chunks_per_block, reps=1, n_stages=4):
    """Build + compile the per-core Bass program. n_chunks_per_block is a
    tuple of BLOCKS ints (shared across cores; data-dependent).  With
    reps>1 the full computation is repeated back-to-back (for benchmarking:
    amortizes per-dispatch launch overhead; every rep recomputes the output
    from DRAM inputs and rewrites y identically)."""
    C_tot = sum(n_chunks_per_block)
    assert C_tot % GROUP == 0
    M = C_tot * CHUNK
    n_groups = C_tot // GROUP

    # chunk index -> block id
    chunk_block = []
    for b, cb in enumerate(n_chunks_per_block):
        chunk_block.extend([b] * cb)

    nc = bacc.Bacc(None, target_bir_lowering=False)
    xyT = nc.dram_tensor("xyT", [2, M], BF16, kind="ExternalInput")
    zno = nc.dram_tensor("zno", [3, M], F16, kind="ExternalInput")
    tkr = nc.dram_tensor("tkr", [3, WIN], F16, kind="ExternalInput")
    w1 = nc.dram_tensor("w1", [2, H], BF16, kind="ExternalInput")
    w2 = nc.dram_tensor("w2", [H, H], BF16, kind="ExternalInput")
    w3 = nc.dram_tensor("w3", [H, S], F16, kind="ExternalInput")
    b1 = nc.dram_tensor("b1", [H, 1], FP, kind="ExternalInput")
    b2 = nc.dram_tensor("b2", [H, 1], FP, kind="ExternalInput")
    y = nc.dram_tensor("y", [S, OUT_W], FP, kind="ExternalOutput")

    with tile.TileContext(nc) as tc:
        with (
            tc.tile_pool(name="consts", bufs=1) as consts,
            tc.tile_pool(name="xyp", bufs=2) as xyp,
            tc.tile_pool(name="znop", bufs=2) as znop,
            tc.tile_pool(name="h1p", bufs=4) as h1p,
            tc.tile_pool(name="h2p", bufs=6) as h2p,
            tc.tile_pool(name="rp", bufs=6) as rp,
            tc.tile_pool(name="gp", bufs=6) as gp,
            tc.tile_pool(name="outp", bufs=2) as outp,
            tc.tile_pool(name="acc", bufs=1, space="PSUM") as accp,
            tc.tile_pool(name="mlppsum", bufs=2, space="PSUM") as mlppsum,
            tc.tile_pool(name="rdpsum", bufs=2, space="PSUM") as rdpsum,
            tc.tile_pool(name="dpsum", bufs=1, space="PSUM") as dpsum,
        ):
            # --- constants, loaded once ---
            w1_s = consts.tile([2, H], BF16)
            nc.sync.dma_start(out=w1_s[:], in_=w1[:])
            w2_s = consts.tile([H, H], BF16)
            nc.sync.dma_start(out=w2_s[:], in_=w2[:])
            w3_s = consts.tile([H, S], F16)
            nc.sync.dma_start(out=w3_s[:], in_=w3[:])
            b1_s = consts.tile([H, 1], FP)
            nc.sync.dma_start(out=b1_s[:], in_=b1[:])
            b2_s = consts.tile([H, 1], FP)
            nc.sync.dma_start(out=b2_s[:], in_=b2[:])
            tkr_s = consts.tile([3, WIN], F16)
            nc.sync.dma_start(out=tkr_s[:], in_=tkr[:])
            zeros_s = consts.tile([1, OUT_W], FP)
            nc.vector.memset(zeros_s[:], 0.0)
            zcol_s = consts.tile([CHUNK, 1], FP)
            nc.vector.memset(zcol_s[:], 0.0)

            def evict_relu(dst, src, bias, use_act):
                if use_act:
                    nc.scalar.activation(
                        dst, src, mybir.ActivationFunctionType.Relu, bias=bias
                    )
                else:
                    nc.vector.tensor_scalar(
                        out=dst, in0=src, scalar1=bias, scalar2=0.0,
                        op0=mybir.AluOpType.add, op1=mybir.AluOpType.max,
                    )

            # ---------- 5-stage software pipeline over groups ----------
            # S0(g): DMA xy, zno           S1(g): mm1, relu1
            # S2(g): mm2, relu2            S3(g): mm3 x8, tanh, rTS,
            #                                     d-mm x8, exp
            # S4(g): einsum x8
            # Each engine's dependencies are >= 1 iteration old, so the
            # per-engine instruction streams never head-of-line block.
            # `acc` and `st` are rebound per rep by the driver loop below;
            # the stage closures read the current binding at call time.
            acc = None
            st = {}  # per-group pipeline state

            DMA_G = 8  # groups fetched per input DMA pair

            def s0(g):
                if g % DMA_G != 0:
                    return
                ng = min(DMA_G, n_groups - g)
                e0 = g * GROUP * CHUNK
                w = ng * GROUP * CHUNK
                xy_t = xyp.tile([2, DMA_G * GROUP * CHUNK], BF16, tag="xy")
                nc.sync.dma_start(out=xy_t[:, 0:w], in_=xyT[:, e0:e0 + w])
                zno_t = znop.tile([3, DMA_G * GROUP * CHUNK], F16, tag="zno")
                nc.sync.dma_start(out=zno_t[:, 0:w], in_=zno[:, e0:e0 + w])
                for j in range(ng):
                    st[g + j] = {"xy": xy_t, "zno": zno_t,
                                 "off": j * GROUP * CHUNK}

            def s1(g):
                s = st[g]
                h1_s = h1p.tile([H, GROUP * CHUNK], BF16, tag="h1")
                s["h1"] = h1_s
                for sb in range(GROUP * CHUNK // BATCH):
                    h1_ps = mlppsum.tile([H, BATCH], FP, tag="mlp")
                    for half in range(2):
                        nc.tensor.matmul(
                            out=h1_ps[:, half * 512:(half + 1) * 512],
                            lhsT=w1_s[:],
                            rhs=s["xy"][:, s["off"] + sb * BATCH + half * 512:
                                        s["off"] + sb * BATCH + (half + 1) * 512],
                            start=True, stop=True,
                        )
                    evict_relu(h1_s[:, sb * BATCH:(sb + 1) * BATCH],
                               h1_ps[:], b1_s[:, 0:1], False)  # DVE

            def s2(g):
                s = st[g]
                h2_s = h2p.tile([H, GROUP * CHUNK], F16, tag="h2")
                s["h2"] = h2_s
                for sb in range(GROUP * CHUNK // BATCH):
                    h2_ps = mlppsum.tile([H, BATCH], FP, tag="mlp")
                    for half in range(2):
                        nc.tensor.matmul(
                            out=h2_ps[:, half * 512:(half + 1) * 512],
                            lhsT=w2_s[:],
                            rhs=s["h1"][:, sb * BATCH + half * 512:
                                        sb * BATCH + (half + 1) * 512],
                            start=True, stop=True,
                        )
                    # column-split eviction: ACT | DVE
                    o = sb * BATCH
                    evict_relu(h2_s[:, o:o + SPLIT], h2_ps[:, 0:SPLIT],
                               b2_s[:, 0:1], True)
                    evict_relu(h2_s[:, o + SPLIT:o + BATCH],
                               h2_ps[:, SPLIT:BATCH], b2_s[:, 0:1], False)
                s["h2"] = h2_s

            def s3(g):
                s = st[g]
                r_s = rp.tile([CHUNK, GROUP * S], F16, tag="r")
                for hh in range(2):
                    resp_ps = rdpsum.tile([CHUNK, GROUP * S // 2], FP, tag="rd")
                    for ci in range(GROUP // 2):
                        cc = hh * (GROUP // 2) + ci
                        nc.tensor.matmul(
                            out=resp_ps[:, ci * S:(ci + 1) * S],
                            lhsT=s["h2"][:, cc * CHUNK:(cc + 1) * CHUNK],
                            rhs=w3_s[:],
                            start=True, stop=True,
                        )
                    # sigmoid(x) = 0.5*tanh(0.5x) + 0.5; tanh on ACT (exp's
                    # table set), then the affine on the idle Pool engine.
                    half = r_s[:, hh * (GROUP // 2) * S:(hh + 1) * (GROUP // 2) * S]
                    nc.scalar.activation(
                        half, resp_ps[:], mybir.ActivationFunctionType.Tanh,
                        scale=0.5,
                    )
                    nc.gpsimd.tensor_scalar(
                        out=half, in0=half, scalar1=0.5, scalar2=0.5,
                        op0=mybir.AluOpType.mult, op1=mybir.AluOpType.add,
                    )
                d_ps = dpsum.tile([CHUNK, GROUP * WIN], FP, tag="d")
                for cc in range(GROUP):
                    last_pe = nc.tensor.matmul(
                        out=d_ps[:, cc * WIN:(cc + 1) * WIN],
                        lhsT=s["zno"][:, s["off"] + cc * CHUNK:
                                      s["off"] + (cc + 1) * CHUNK],
                        rhs=tkr_s[:],
                        start=True, stop=True,
                    )
                g_s = gp.tile([CHUNK, GROUP * WIN], F16, tag="g")
                nc.scalar.activation(
                    g_s[:], d_ps[:], mybir.ActivationFunctionType.Exp,
                    bias=zcol_s[:, 0:1],
                )
                s["r"] = r_s
                s["g"] = g_s
                s["last_pe"] = last_pe

            def s4(g):
                from concourse.tile_rust import add_dep_helper
                s = st.pop(g)
                first = True
                for cc in range(GROUP):
                    b = chunk_block[g * GROUP + cc]
                    w = b * BLOCK_TICKS
                    last = g == n_groups - 1 and cc == GROUP - 1
                    mm = nc.tensor.matmul(
                        out=acc[0:S, w:w + WIN],
                        lhsT=s["r"][:, cc * S:(cc + 1) * S],
                        rhs=s["g"][:, cc * WIN:(cc + 1) * WIN],
                        start=False, stop=last,
                        skip_group_check=True,
                    )
                    if first and (g + 2) in st and "last_pe" in st[g + 2]:
                        # order-only edge: keep einsum(g) after group g+2's
                        # matmul phase so PE never head-of-line blocks on
                        # the tanh/exp chain of group g.
                        add_dep_helper(
                            mm.ins, st[g + 2]["last_pe"].ins,
                            sync=False, reason="einsum skew",
                        )
                    first = False

            # einsum gets a 2-iteration skew (offset 5, not 4) so the
            # tanh -> Pool rTS chain of group g completes a full iteration
            # before PE reaches einsum(g) in its stream.
            stage_offsets = [(s0, 0), (s1, 1), (s2, 2), (s3, 3), (s4, 5)]
            stage_offsets = stage_offsets[:n_stages + 1]
            for _rep in range(reps):
                acc = accp.tile([S, OUT_W], FP, tag="acc")
                nc.tensor.matmul(
                    out=acc[:],
                    lhsT=zeros_s[0:1, 0:S],
                    rhs=zeros_s[0:1, 0:OUT_W],
                    start=True,
                    stop=False,
                    skip_group_check=True,
                )
                st = {}
                for it in range(n_groups + 6):
                    for fn, off in stage_offsets:
                        g = it - off
                        if 0 <= g < n_groups:
                            fn(g)

                out_sb = outp.tile([S, OUT_W], FP, tag="out")
                nc.vector.tensor_copy(out=out_sb[:], in_=acc[:])
                nc.sync.dma_start(out=y[:], in_=out_sb[:])

    nc.compile()
    return nc


_CACHE = {}


def _get_nc(n_chunks_per_block, reps=1, n_stages=4):
    key = (tuple(n_chunks_per_block), reps, n_stages)
    if key not in _CACHE:
        _CACHE[key] = _build_nc(key[0], reps=reps, n_stages=n_stages)
    return _CACHE[key]


def _prep_inputs(el_photons, xy_positions, z_positions):
    """Shard by z-range, bucket into 16-tick blocks, pad, build per-core
    device arrays."""
    el = np.asarray(el_photons, np.float32).reshape(-1)
    xy = np.asarray(xy_positions, np.float32)
    z = np.asarray(z_positions, np.float32).reshape(-1)

    core = np.clip((z // CORE_TICKS).astype(np.int64), 0, N_CORES - 1)
    zrel = z - core * CORE_TICKS
    block = np.clip((zrel // BLOCK_TICKS).astype(np.int64), 0, BLOCKS - 1)
    # block-relative z', and the per-electron exponent constant
    zp = (zrel - (block * BLOCK_TICKS + BLOCK_TICKS // 2)).astype(np.float32)
    wexp = np.where(
        el > 0,
        (-0.5 * zp.astype(np.float64) ** 2
         + np.log(np.maximum(el, 1e-45).astype(np.float64) * C_GAUSS)),
        NEG_BIG,
    ).astype(np.float32)

    counts = np.zeros((N_CORES, BLOCKS), np.int64)
    np.add.at(counts, (core, block), 1)
    cpb = np.ceil(counts.max(axis=0) / CHUNK).astype(np.int64)  # chunks per block
    C_tot = int(cpb.sum())
    pad_chunks = (-C_tot) % GROUP
    cpb[0] += pad_chunks
    C_tot += pad_chunks
    M = C_tot * CHUNK

    order = np.lexsort((block, core))  # stable sort by (core, block)
    el_o, xy_o, zp_o, w_o, blk_o, core_o = (
        el[order], xy[order], zp[order], wexp[order], block[order], core[order]
    )

    block_starts = np.concatenate(([0], np.cumsum(cpb)[:-1])) * CHUNK

    per_core = []
    for k in range(N_CORES):
        xyT_k = np.zeros((2, M), np.float32)
        zno_k = np.empty((3, M), np.float16)
        zno_k[0] = 0.0         # padded electrons: z' = 0
        zno_k[1] = -0.5
        zno_k[2] = NEG_BIG     # padded electrons contribute exp(-1e30) = 0
        sel = core_o == k
        xy_sel = xy_o[sel]
        zp_sel = zp_o[sel]
        w_sel = w_o[sel]
        blk_sel = blk_o[sel]
        bcounts = np.bincount(blk_sel, minlength=BLOCKS)
        src = 0
        for b in range(BLOCKS):
            nb = int(bcounts[b])
            dst = int(block_starts[b])
            xyT_k[:, dst:dst + nb] = xy_sel[src:src + nb].T
            zno_k[0, dst:dst + nb] = zp_sel[src:src + nb]
            zno_k[2, dst:dst + nb] = w_sel[src:src + nb]
            src += nb
        per_core.append((xyT_k.astype(ml_dtypes.bfloat16), zno_k))

    # tick rows, block-relative: t' in [-16, 16)
    tp = np.arange(WIN, dtype=np.float32) - (BLOCK_TICKS // 2 + MARGIN)
    tkr = np.stack([tp, tp * tp, np.ones(WIN, np.float32)]).astype(np.float16)
    return tuple(int(c) for c in cpb), per_core, tkr


def kernel(el_photons, xy_positions, z_positions, W1, b1, W2, b2, W3, b3):
    b3 = np.asarray(b3, np.float32)
    assert np.allclose(b3, 0.0), "kernel assumes b3 == 0"

    cpb, per_core, tkr = _prep_inputs(el_photons, xy_positions, z_positions)
    nc = _get_nc(cpb)

    shared = {
        "tkr": tkr,
        "w1": np.asarray(W1, np.float32).astype(__import__("ml_dtypes").bfloat16),
        "w2": np.asarray(W2, np.float32).astype(__import__("ml_dtypes").bfloat16),
        "w3": np.asarray(W3, np.float16),
        "b1": np.asarray(b1, np.float32).reshape(H, 1),
        "b2": np.asarray(b2, np.float32).reshape(H, 1),
    }
    in_maps = []
    for k in range(N_CORES):
        xyT_k, zno_k = per_core[k]
        in_maps.append({"xyT": xyT_k, "zno": zno_k, **shared})

    res = run_bass_kernel_spmd(nc, in_maps, core_ids=list(range(N_CORES)))

    out = np.zeros((S, T_TICKS), np.float64)
    for k in range(N_CORES):
        yk = res.results[k]["y"].astype(np.float64)
        lo = k * CORE_TICKS - MARGIN
        j0 = max(0, -lo)
        j1 = min(OUT_W, T_TICKS - lo)
        out[:, lo + j0:lo + j1] += yk[:, j0:j1]
    return out.astype(np.float32)

